# revision 17
# baseline (speedup 1.0000x reference)
"""Multi-head attention forward on 8 Trainium2 NeuronCores (Bass/Tile).

Problem: B=2, N=2048, D=1024, H=16 heads of dh=64, fp32.

Sharding: tensor-parallel over heads — core c owns heads {2c, 2c+1} for both
batches through projections + attention; per-window AllToAll re-shards by
token so each core computes the output projection (full Wo) for its 512
tokens, interleaved into the attention stream.

Key structure vs the naive version:
  - activations travel as [feature, token]; every contraction is on the
    partition axis.
  - score matmuls: the two heads' K=64 matmuls are packed into disjoint
    PE row-groups via tile_position (0,0)/(64,0) and run concurrently.
  - exp is split across engines: head A on ScalarE (true Exp), head B on
    VectorE via Schraudolph bit-trick (int32 out, bitcast to f32 for the PE).
  - attn@v: lhsT = v_aug [m, 65] with a ones column so PSUM row 64
    accumulates softmax denominators for free.
  - normalization: selector matmul broadcasts 1/denom across partitions.
  - 8 x 512-token windows, each followed by its own AllToAll (bf16) whose
    out-projection runs two windows later, inside the PE stream.
"""
from contextlib import ExitStack

import numpy as np
import ml_dtypes

import concourse.bass as bass
import concourse.tile as tile
from concourse import bacc, mybir
from concourse.bass_utils import run_bass_kernel_spmd
from concourse.masks import make_identity

F32 = mybir.dt.float32
F32R = mybir.dt.float32r
BF16 = mybir.dt.bfloat16
I16 = mybir.dt.int16

B, N, D, H, DH = 2, 2048, 1024, 16, 64
W = 8                    # cores
TOK = B * N              # 4096 flattened tokens
NW = 8                   # 512-token attention windows
WTOK = TOK // NW         # 512
MCB = N // 128           # m-chunks per batch (16)
KC = D // 128            # contraction chunks for projections

A_EXP = 184.664965     # 2^7 / ln 2 (bf16 Schraudolph)
C_EXP = 16250.75       # 127*128 - 5.25 (bf16 Schraudolph, ~3.3% max rel)

_CACHE = {}


def build_bass():
    nc = bacc.Bacc("TRN2", target_bir_lowering=False)

    xT_d = nc.declare_dram_parameter("xT", [D, TOK], BF16, isOutput=False)
    wq_d = nc.declare_dram_parameter("wq", [D, 128], BF16, isOutput=False)
    wk_d = nc.declare_dram_parameter("wk", [D, 128], BF16, isOutput=False)
    wv_d = nc.declare_dram_parameter("wv", [D, 128], BF16, isOutput=False)
    wo_d = nc.declare_dram_parameter("wo", [D, D], BF16, isOutput=False)
    bqkv_d = nc.declare_dram_parameter("bqkv", [128, 3], F32, isOutput=False)
    out_d = nc.declare_dram_parameter("out", [NW * 64, D], F32, isOutput=True)

    a2a_in = [nc.dram_tensor(f"a2a_in{w}", [W, 128, 64], BF16) for w in range(NW)]
    a2a_out = [nc.dram_tensor(f"a2a_out{w}", [W, 128, 64], BF16) for w in range(NW)]

    TC = TOK // 512      # 512-token chunks in stage 1 (8)

    with tile.TileContext(nc) as tc, ExitStack() as ctx:
        sb1 = ctx.enter_context(tc.tile_pool(name="sb1", bufs=1))
        sbe = ctx.enter_context(tc.tile_pool(name="sbe", bufs=2))
        stage1 = ExitStack()
        sbw = stage1.enter_context(tc.tile_pool(name="sbw", bufs=1))
        sbx = stage1.enter_context(tc.tile_pool(name="sbx", bufs=2))
        ps_pj = stage1.enter_context(tc.tile_pool(name="ps_pj", bufs=2, space="PSUM"))

        # ---------- weights (scalar-engine HWDGE queue; x goes on sync) ----
        wq = sbw.tile([128, KC, 128], BF16, tag="wq")
        wk = sbw.tile([128, KC, 128], BF16, tag="wk")
        wv = sbw.tile([128, KC, 128], BF16, tag="wv")

        # ---------- constants ----------
        ident_f = sb1.tile([128, 128], F32, tag="ident_f")
        make_identity(nc, ident_f[:])
        ident = sb1.tile([128, 128], BF16, tag="ident")
        nc.vector.tensor_copy(ident[:], ident_f[:])



        sel_f = sb1.tile([33, 128], F32, tag="sel_f")
        nc.vector.memset(sel_f[:], 0.0)
        nc.vector.memset(sel_f[0:1, 0:64], 1.0)
        nc.vector.memset(sel_f[32:33, 64:128], 1.0)
        sel = sb1.tile([33, 128], BF16, tag="sel")
        nc.vector.tensor_copy(sel[:], sel_f[:])

        bias = sb1.tile([128, 3], F32, tag="bias")

        # ---------- stage 1: projections ----------
        qT = sb1.tile([128, TOK], BF16, tag="qT")
        kT = sb1.tile([128, TOK], BF16, tag="kT")
        v_aug = sb1.tile([128, 2 * MCB, 130], BF16, tag="v_aug")
        nc.vector.memset(v_aug[:, :, 64:65], 1.0)
        nc.vector.memset(v_aug[:, :, 129:130], 1.0)

        for tp2 in range(TC // 2):
            ta, tb = 2 * tp2, 2 * tp2 + 1
            xta = sbx.tile([128, KC, 512], BF16, tag="xta")
            xtb = sbx.tile([128, KC, 512], BF16, tag="xtb")
            for k in range(KC):
                if tp2 == 0:
                    nc.scalar.dma_start(wv[:, k, :], wv_d[bass.ts(k, 128), :])
                nc.sync.dma_start(xta[:, k, :],
                                  xT_d[bass.ts(k, 128), bass.ts(ta, 512)])
            for k in range(KC):
                nc.sync.dma_start(xtb[:, k, :], xT_d[bass.ts(k, 128), bass.ts(tb, 512)])
            if tp2 == 0:
                for k in range(KC):
                    nc.scalar.dma_start(wq[:, k, :], wq_d[bass.ts(k, 128), :])
                    nc.scalar.dma_start(wk[:, k, :], wk_d[bass.ts(k, 128), :])
                nc.sync.dma_start(bias[:], bqkv_d[:])

            tsla, tslb = bass.ts(ta, 512), bass.ts(tb, 512)
            pja = ps_pj.tile([128, 512], F32, tag="pj0")
            pjb = ps_pj.tile([128, 512], F32, tag="pj1")
            for k in range(KC):
                nc.tensor.matmul(pja[:], wv[:, k, :], xta[:, k, :],
                                 start=(k == 0), stop=(k == KC - 1))
                nc.tensor.matmul(pjb[:], wv[:, k, :], xtb[:, k, :],
                                 start=(k == 0), stop=(k == KC - 1))
            vts = []
            for t, pj in ((ta, pja), (tb, pjb)):
                vt = sbx.tile([128, 512], BF16, tag=f"vt{t % 2}")
                nc.vector.tensor_scalar_add(vt[:], pj[:], bias[:, 2:3])
                vts.append((t, vt))
            for t, vt in vts:
                for i in range(4):
                    gm = 4 * t + i
                    tp = ps_pj.tile([128, 128], BF16, tag="tp")
                    nc.tensor.transpose(tp[:], vt[:, bass.ts(i, 128)], ident[:])
                    nc.vector.tensor_copy(v_aug[:, gm, 0:64], tp[:, 0:64])
                    nc.vector.tensor_copy(v_aug[:, gm, 65:129], tp[:, 64:128])

            pja = ps_pj.tile([128, 512], F32, tag="pj0")
            pjb = ps_pj.tile([128, 512], F32, tag="pj1")
            for k in range(KC):
                nc.tensor.matmul(pja[:], wq[:, k, :], xta[:, k, :],
                                 start=(k == 0), stop=(k == KC - 1))
                nc.tensor.matmul(pjb[:], wq[:, k, :], xtb[:, k, :],
                                 start=(k == 0), stop=(k == KC - 1))
            nc.vector.tensor_scalar_add(qT[:, tsla], pja[:], bias[:, 0:1])
            nc.vector.tensor_scalar_add(qT[:, tslb], pjb[:], bias[:, 0:1])

            pja = ps_pj.tile([128, 512], F32, tag="pj0")
            pjb = ps_pj.tile([128, 512], F32, tag="pj1")
            for k in range(KC):
                nc.tensor.matmul(pja[:], wk[:, k, :], xta[:, k, :],
                                 start=(k == 0), stop=(k == KC - 1))
                nc.tensor.matmul(pjb[:], wk[:, k, :], xtb[:, k, :],
                                 start=(k == 0), stop=(k == KC - 1))
            nc.vector.tensor_scalar_add(kT[:, tsla], pja[:], bias[:, 1:2])
            nc.vector.tensor_scalar_add(kT[:, tslb], pjb[:], bias[:, 1:2])

        stage1.close()
        sb3 = ctx.enter_context(tc.tile_pool(name="sb3", bufs=1))
        wo = sb3.tile([128, KC, D], BF16, tag="wo")
        nc.sync.dma_start(wo[:], wo_d[:].rearrange("(k p) c -> p k c", p=128))

        # ---------- stage 2: attention + interleaved out-projection -------
        ps_sc = ctx.enter_context(tc.tile_pool(name="ps_sc", bufs=2, space="PSUM"))
        ps_ha = ctx.enter_context(tc.tile_pool(name="ps_ha", bufs=1, space="PSUM"))
        ps_op = ctx.enter_context(tc.tile_pool(name="ps_op", bufs=2, space="PSUM"))
        rcp = sb1.tile([33, 512], BF16, tag="rcp")
        nc.vector.memset(rcp[:], 0.0)

        def emit_window_end1(w, ha0, ha1):
            # drain ha PSUM, stage denominators, kick the selector matmul
            hs0 = sbe.tile([65, 512], F32, tag="hs0", bufs=1)
            hs1 = sbe.tile([128, 512], F32, tag="hs1", bufs=1)
            nc.vector.tensor_copy(hs0[:], ha0[:])
            nc.vector.tensor_copy(hs1[64:128, :], ha1[0:64, :])
            nc.vector.tensor_copy(rcp[0:1, :], hs0[64:65, :])
            nc.vector.tensor_copy(rcp[32:33, :], ha1[64:65, :])
            return (hs0, hs1)

        def emit_window_end2(w, pend):
            # normalize into bf16 staging, ship this window's AllToAll
            hs0, hs1 = pend
            bc = ps_sc.tile([128, 512], F32, tag="scA")
            nc.tensor.matmul(bc[:], sel[:], rcp[:], start=True, stop=True)
            bcr = sbe.tile([128, 512], F32, tag="bcr", bufs=1)
            nc.vector.reciprocal_approx_fast(bcr[:], bc[:])
            hstg = sbe.tile([128, W, 64], BF16, tag="hstg", bufs=1)
            nc.vector.tensor_mul(hstg[0:64, :, :].rearrange("p j t -> p (j t)"),
                                 hs0[0:64, :], bcr[0:64, :])
            nc.vector.tensor_mul(hstg[64:128, :, :].rearrange("p j t -> p (j t)"),
                                 hs1[64:128, :], bcr[64:128, :])
            nc.sync.dma_start(a2a_in[w][:].rearrange("j p t -> p j t"), hstg[:])
            nc.gpsimd.collective_compute(
                "AllToAll",
                mybir.AluOpType.bypass,
                ins=[a2a_in[w][:]],
                outs=[a2a_out[w][:]],
                replica_groups=[list(range(W))],
            )

        hT_tiles = {}

        def prefetch_hT(w):
            hT = sb3.tile([128, KC, 64], BF16, tag="hT", bufs=3)
            nc.sync.dma_start(hT[:], a2a_out[w][:].rearrange("j p t -> p j t"))
            hT_tiles[w] = hT

        def emit_outproj(w):
            hT = hT_tiles.pop(w)
            op = ps_op.tile([128, 512], F32, tag="op")
            for k in range(KC):
                nc.tensor.matmul(op[0:64, :], hT[:, k, :], wo[:, k, 0:512],
                                 start=(k == 0), stop=(k == KC - 1),
                                 tile_position=(0, 0))
                nc.tensor.matmul(op[64:128, :], hT[:, k, :], wo[:, k, 512:1024],
                                 start=(k == 0), stop=(k == KC - 1),
                                 tile_position=(0, 64))
            ot = sb3.tile([64, 1024], F32, tag="ot", bufs=2)
            nc.vector.tensor_copy(ot[:, 0:512], op[0:64, :])
            nc.vector.tensor_copy(ot[:, 512:1024], op[64:128, :])
            nc.sync.dma_start(out_d[bass.ts(w, 64), :], ot[:])

        from collections import deque
        pipe = deque()  # (eA, eB_bf16_ap, gm, ha0, ha1, w, mc)

        def emit_av(pr):
            eA, eB, gm, ha0, ha1, w, mc = pr
            nc.tensor.matmul(ha0[:], v_aug[:, gm, 0:65], eA[:],
                             start=(mc == 0), stop=(mc == MCB - 1))
            nc.tensor.matmul(ha1[:], v_aug[:, gm, 65:130], eB,
                             start=(mc == 0), stop=(mc == MCB - 1))

        pending_we = []   # [(w, pend)]

        def retire_one():
            pr = pipe.popleft()
            emit_av(pr)
            pw = pr[5]
            if pr[6] == MCB - 1:
                pending_we.append((pw, emit_window_end1(pw, pr[3], pr[4])))
            elif pr[6] == 2 and pending_we:
                w2, pend = pending_we.pop(0)
                emit_window_end2(w2, pend)
                if w2 >= 1:
                    prefetch_hT(w2 - 1)
                if 2 <= w2 <= 4:
                    emit_outproj(w2 - 2)

        for w in range(NW):
            b = w // 4
            nsl = bass.ts(w, 512)
            ha0 = ps_ha.tile([65, 512], F32, tag="ha0")
            ha1 = ps_ha.tile([65, 512], F32, tag="ha1")
            for mc in range(MCB):
                gm = MCB * b + mc
                msl = bass.ts(gm, 128)
                scA = ps_sc.tile([128, 512], F32, tag="scA")
                scB = ps_sc.tile([128, 512], F32, tag="scB")
                nc.tensor.matmul(scA[:], kT[0:64, msl], qT[0:64, nsl],
                                 start=True, stop=True, tile_position=(0, 0))
                nc.tensor.matmul(scB[:], kT[64:128, msl], qT[64:128, nsl],
                                 start=True, stop=True, tile_position=(64, 0))
                if len(pipe) == 3:
                    retire_one()
                eA = sbe.tile([128, 512], BF16, tag="eA", bufs=4)
                nc.scalar.activation(eA[:], scA[:],
                                     mybir.ActivationFunctionType.Exp)
                if mc % 2 == 0:
                    # ScalarE takes head B on even chunks
                    eBt = sbe.tile([128, 512], BF16, tag="eBs", bufs=3)
                    nc.scalar.activation(eBt[:], scB[:],
                                         mybir.ActivationFunctionType.Exp)
                    eB_ap = eBt[:]
                else:
                    # DVE Schraudolph on odd chunks
                    eBt = sbe.tile([128, 512], I16, tag="eBi", bufs=3)
                    nc.vector.tensor_scalar(eBt[:], scB[:], A_EXP, C_EXP,
                                            mybir.AluOpType.mult,
                                            mybir.AluOpType.add)
                    eB_ap = eBt[:].bitcast(BF16)
                pipe.append((eA, eB_ap, gm, ha0, ha1, w, mc))

        # epilogue: drain the pipeline, then finish windows 5-7.
        # in-loop end2 has run for windows 0-6 (outproj 0-4, prefetch 0-5).
        retire_one()
        retire_one()
        retire_one()
        w2, pend = pending_we.pop(0)
        emit_window_end2(w2, pend)          # window 7: ships last a2a
        emit_outproj(3)
        emit_outproj(4)
        prefetch_hT(NW - 2)
        emit_outproj(5)
        prefetch_hT(NW - 1)
        emit_outproj(NW - 2)
        emit_outproj(NW - 1)

    nc.compile()
    return nc


def _prep_inputs(x, Wq, bq, Wk, bk, Wv, bv, Wo, bo):
    bfc = lambda a: np.ascontiguousarray(a).astype(ml_dtypes.bfloat16)
    xT = bfc(x.reshape(TOK, D).T)
    wo_bf = bfc(Wo)
    in_maps = []
    for c in range(W):
        sl = slice(128 * c, 128 * (c + 1))
        bqkv = np.stack([bq[sl] / 8.0, bk[sl], bv[sl]], axis=1).astype(np.float32)
        in_maps.append({
            "xT": xT,
            "wq": bfc(Wq[:, sl] / 8.0),
            "wk": bfc(Wk[:, sl]),
            "wv": bfc(Wv[:, sl]),
            "wo": wo_bf,
            "bqkv": np.ascontiguousarray(bqkv),
        })
    return in_maps


def run(x, Wq, bq, Wk, bk, Wv, bv, Wo, bo, **run_kwargs):
    if "nc" not in _CACHE:
        _CACHE["nc"] = build_bass()
    nc = _CACHE["nc"]
    in_maps = _prep_inputs(x, Wq, bq, Wk, bk, Wv, bv, Wo, bo)
    res = run_bass_kernel_spmd(nc, in_maps, list(range(W)), **run_kwargs)
    # core c, row 64*w + i  ->  global token 512*w + 64*c + i
    out = np.empty((NW, W, 64, D), np.float32)
    for c in range(W):
        out[:, c] = res.results[c]["out"].reshape(NW, 64, D)
    out = out.reshape(TOK, D).reshape(B, N, D) + bo.astype(np.float32)
    return out.astype(np.float32), res


def kernel(x, Wq, bq, Wk, bk, Wv, bv, Wo, bo):
    x, Wq, bq, Wk, bk, Wv, bv, Wo, bo = (
        np.asarray(a, dtype=np.float32)
        for a in (x, Wq, bq, Wk, bk, Wv, bv, Wo, bo)
    )
    out, _ = run(x, Wq, bq, Wk, bk, Wv, bv, Wo, bo)
    return out
